# revision 26
# baseline (speedup 1.0000x reference)
"""AdaptiveConstantEmbeddings distributed Bass kernel for one TRN2 chip.

Reference semantics per domain g (two independent domains):
    e        = max(0, idx - C)                       # [B,S] adaptive row ids
    emb      = adapt_table[e]                        # [B,S,D]
    rel      = emb @ const_table.T                   # [B,S,C]
    out[b,s] = const_table rows where rel == rowmax  # top-1 retrieval

Key algebra: rel rows only depend on e, so compute R = adapt @ const.T
once per domain ([A,C]), argmax over C per adaptive row, then
out[b,s] = const_table[best[e[b,s]]] is a pure gather.

v3 changes vs the AllGather-pipelined v2 (230us):
  * ZERO collectives.  v2 AllGather'd the per-core G shards so each core
    could emit its own batches' tokens; the 4 pipelined AGs ran at only
    24-48 GB/s bus and cost ~116us of wall time including a 40us tail.
    v3 instead re-shards the OUTPUT by adaptive-row range: core r of a
    domain group owns rows [r*1024,(r+1)*1024) AND emits exactly the
    domain tokens whose e lands in that range (the host buckets tokens
    per core; e==0 tokens are filled host-side from the exported row0).
    Every gather is then purely local.
  * Matmuls run bf16 3-term hi/lo (R = Ah@Bh + Ah@Bl + Al@Bh; verified 0
    argmax flips) with ONE standalone LDWEIGHTS per 4-8 matmuls
    (weight-block-major), instead of a fused LDWEIGHTS per matmul.
  * Tables stream in over 4 DMA queues (sync/scalar/vector/gpsimd).
  * Per a-tile T: argmax -> indirect-gather const[best] -> g_t[T] in
    DRAM -> token dma_gather for tile T's bucket -> out DMA, all
    pipelined behind the next tiles' matmuls.

Sharding (8 cores, expert-style): cores 0-3 own domain 0, cores 4-7 own
domain 1.  Within a 4-core group the A=4096 adaptive rows split
1024/core for matmul+argmax, and each core outputs the tokens of its
own row range (~2050 of 16384, padded to 8 chunks of CAP).
"""

import numpy as np

from concourse import bacc, bass, mybir, tile
from concourse.bass_utils import run_bass_kernel_spmd

F32 = mybir.dt.float32
BF16 = mybir.dt.bfloat16
I32 = mybir.dt.int32
I16 = mybir.dt.int16
U16 = mybir.dt.uint16

B, S = 16, 1024
C = 4096          # codebook rows per domain
A = 4096          # adaptive rows per domain
D = 256           # embedding dim
NCORES = 8
GSIZE = 4                     # cores per domain group
ASH = A // GSIZE              # 1024 adaptive rows per core
ATILES = ASH // 128           # 8
KCH = D // 128                # 2 contraction chunks
CW = 512                      # psum tile width (one bank per matmul)
CTILES = C // CW              # 8 psum column tiles
CAP0 = 384                    # default tokens per tile-chunk (3*128)

_NC_CACHE = {}


def _build(cap=CAP0, bare=True):
    nc = bacc.Bacc("TRN2", target_bir_lowering=False, debug=False, num_devices=NCORES)

    ncol = ATILES * cap // 128          # out columns (tokens = col*128 + part)

    # hi/lo bf16 split of [adapt_shard.T | const.T]; one DMA per k-chunk
    tabsH = nc.declare_dram_parameter("tabsH", [D, ASH + C], BF16, isOutput=False)
    tabsL = nc.declare_dram_parameter("tabsL", [D, ASH + C], BF16, isOutput=False)
    constN = nc.declare_dram_parameter("constN", [C, D], F32, isOutput=False)
    # wrapped dma_gather indices (tile-local row ids, 0..127):
    # eidx16[q, s] = e''[s*16 + q%16], replicated across the eight
    # 16-partition groups; chunk T uses columns [T*cap/16, (T+1)*cap/16)
    eidx16 = nc.declare_dram_parameter("eidx16", [128, ATILES * cap // 16], I16,
                                       isOutput=False)
    # out[p, j, :] = row of gathered token j*128 + p (host unpermutes)
    out = nc.declare_dram_parameter("out", [128, ncol, D], F32, isOutput=True)
    # G row 0 of this shard (cores 0/4: the row shared by all e==0 tokens)
    row0 = nc.declare_dram_parameter("row0", [1, D], F32, isOutput=True)

    g_t = [nc.dram_tensor(f"g_t{T}", [128, D], F32) for T in range(ATILES)]

    with tile.TileContext(nc) as tc:
        with (
            tc.tile_pool(name="tabs", bufs=1) as tabs_pool,
            tc.tile_pool(name="work", bufs=3) as work,
            tc.tile_pool(name="small", bufs=4) as small,
            tc.tile_pool(name="ps", bufs=8, space="PSUM") as ps,
            tc.tile_pool(name="gather", bufs=2) as gpool,
        ):
            # tabs[hl][k]: [128, ASH+C] bf16
            tabs = [[tabs_pool.tile([128, ASH + C], BF16, name=f"tabs{hl}{k}")
                     for k in range(KCH)] for hl in range(2)]
            srcs = [tabsH, tabsL]
            load_eng = [nc.sync, nc.scalar, nc.gpsimd]
            NQ = len(load_eng)
            e16 = gpool.tile([128, ATILES * cap // 16], I16, name="e16",
                             tag="e16", bufs=1)
            load_insts = [nc.gpsimd.dma_start(e16[:], eidx16[:])]
            li = 1
            # adaptive shard first (lhsT for the first matmuls)
            for hl in range(2):
                for k in range(KCH):
                    load_insts.append(load_eng[li % NQ].dma_start(
                        tabs[hl][k][:, :ASH],
                        srcs[hl][k * 128:(k + 1) * 128, :ASH]))
                    li += 1
            # const bank pairs, both hi+lo per pair before the next pair
            for c in range(CTILES // 2):
                for hl in range(2):
                    for k in range(KCH):
                        load_insts.append(load_eng[li % NQ].dma_start(
                            tabs[hl][k][:, ASH + c * 1024: ASH + (c + 1) * 1024],
                            srcs[hl][k * 128:(k + 1) * 128,
                                     ASH + c * 1024: ASH + (c + 1) * 1024],
                        ))
                        li += 1
            for i in range(NQ, len(load_insts)):
                tile.add_dep_helper(load_insts[i].ins, load_insts[i - NQ].ins,
                                    False, "load order")

            g_insts, o_insts = [], []
            pending_rows = []
            H, L = 0, 1

            for T in range(ATILES):
                psums = [ps.tile([128, CW], F32, name=f"ps{T}_{c}", tag="ps")
                         for c in range(CTILES)]
                # 3-term bf16: Ah@Bh + Ah@Bl + Al@Bh, weight-block-major so
                # consecutive matmuls share lhsT; 4 banks per half so the
                # other half's PSUM copies overlap.
                for half in range(2):
                    cs = range(half * 4, half * 4 + 4)
                    for (hl, k) in ((H, 0), (H, 1), (L, 0), (L, 1)):
                        lhsT = tabs[hl][k][:, T * 128:(T + 1) * 128]
                        rhs_hls = (H, L) if hl == H else (H,)
                        for rhl in rhs_hls:
                            for c in cs:
                                rhs = tabs[rhl][k][:, ASH + c * CW:
                                                   ASH + (c + 1) * CW]
                                st = (hl == H and k == 0 and rhl == H)
                                sp = (hl == L and k == KCH - 1)
                                nc.tensor.matmul(psums[c][:], lhsT=lhsT,
                                                 rhs=rhs, start=st, stop=sp)

                # argmax via independent halves; left wins exact ties, which
                # matches max_index's first-occurrence rule on the full row.
                # The left half's FIND fires as soon as banks 0-3 are copied,
                # overlapping the right half's copies.
                # copies on scalar, per-bank top-8 on vector, then one global
                # top-8 over the bank top-8s and one max_index over the full
                # row.  max() returns the top-8 values in descending order,
                # so m8[:,0] is the true row max and i8[:,0] its
                # first-occurrence index (reference tie rule).
                r_sb = work.tile([128, C], F32, name=f"r{T}", tag="r")
                m8all = small.tile([128, 64], F32, name=f"m8a_{T}", tag="m8a")
                for c in range(CTILES):
                    nc.scalar.copy(r_sb[:, c * CW:(c + 1) * CW], psums[c][:])
                    nc.vector.max(out=m8all[:, c * 8:(c + 1) * 8],
                                  in_=r_sb[:, c * CW:(c + 1) * CW])
                m8 = small.tile([128, 8], F32, name=f"m8_{T}", tag="m8")
                nc.vector.max(out=m8[:], in_=m8all[:])
                i8 = small.tile([128, 8], U16, name=f"i8_{T}", tag="i8")
                nc.vector.max_index(out=i8[:], in_max=m8[:],
                                    in_values=r_sb[:, :])
                best32 = small.tile([128, 1], I32, name=f"b32_{T}", tag="b32")
                nc.vector.tensor_copy(best32[:], i8[:, :1])

                # G rows for this tile: const[best[a], :]
                g_tile = small.tile([128, D], F32, name=f"g{T}", tag="g")
                nc.gpsimd.indirect_dma_start(
                    out=g_tile[:],
                    out_offset=None,
                    in_=constN[:, :],
                    in_offset=bass.IndirectOffsetOnAxis(ap=best32[:], axis=0),
                )
                gt_dma = nc.sync.dma_start(g_t[T][:, :], g_tile[:])
                if T == 0:
                    nc.scalar.dma_start(row0[:, :], g_tile[0:1, :])
                # out trigger for the PREVIOUS tile, one tile late so its
                # gather has already landed and the sync queue never blocks
                # (an out trigger emitted with its own tile waits ~5us on the
                # gather and stalls the next g_t write; and the tile
                # scheduler hoists triggers as soon as deps allow, so pin
                # them behind this tile's g_t write instead)
                if pending_rows:
                    Tp, prows = pending_rows.pop(0)
                    oi = nc.sync.dma_start(
                        out[:, Tp * (cap // 128):(Tp + 1) * (cap // 128), :],
                        prows[:])
                    tile.add_dep_helper(oi.ins, gt_dma.ins, False, "o after gt")
                    o_insts.append(oi)

                # token gather for this tile's bucket (pads point at row 0
                # of the tile; host ignores pad positions)
                rows = gpool.tile([128, cap // 128, D], F32,
                                  name=f"rows{T}", tag=f"rows{T}", bufs=1)
                gi = nc.gpsimd.dma_gather(
                    out_ap=rows[:],
                    in_ap=g_t[T][:, :],
                    idxs_ap=e16[:, T * (cap // 16):(T + 1) * (cap // 16)],
                    num_idxs=cap,
                    num_idxs_reg=cap,
                    elem_size=D,
                    single_packet=False,
                )
                if g_insts:
                    tile.add_dep_helper(gi.ins, g_insts[-1].ins, False, "g order")
                g_insts.append(gi)
                pending_rows.append((T, rows))

            # remaining out-DMA triggers (last tile's, on sync)
            for Tp, prows in pending_rows:
                oi = nc.sync.dma_start(
                    out[:, Tp * (cap // 128):(Tp + 1) * (cap // 128), :],
                    prows[:])
                if o_insts:
                    tile.add_dep_helper(oi.ins, o_insts[-1].ins, False, "o order")
                o_insts.append(oi)
    nc.compile()
    return nc


def _get_nc(cap, bare=True):
    key = (cap, bare)
    if key not in _NC_CACHE:
        _NC_CACHE[key] = _build(cap, bare)
    return _NC_CACHE[key]


def _bf16_split(x):
    import ml_dtypes
    hi = x.astype(ml_dtypes.bfloat16)
    lo = (x - hi.astype(np.float32)).astype(ml_dtypes.bfloat16)
    return hi, lo


def _in_maps(idx0, idx1, const_table0, const_table1, adapt_table0, adapt_table1):
    idx = [np.asarray(idx0), np.asarray(idx1)]
    const = [np.ascontiguousarray(np.asarray(const_table0, dtype=np.float32)),
             np.ascontiguousarray(np.asarray(const_table1, dtype=np.float32))]
    adapt = [np.asarray(adapt_table0, dtype=np.float32),
             np.asarray(adapt_table1, dtype=np.float32)]
    constT = [np.ascontiguousarray(c.T) for c in const]
    e_dom = [np.maximum(idx[g].reshape(-1).astype(np.int64) - C, 0)
             for g in range(2)]                       # [B*S] per domain

    # capacity: max tokens in any core's 128-row tile bucket, padded to 128
    cap = CAP0
    for g in range(2):
        nz = e_dom[g][e_dom[g] > 0]
        tc_ = np.bincount(nz // 128, minlength=A // 128)
        need = int(np.ceil(max(tc_.max(), 1) / 128) * 128)
        cap = max(cap, need)

    maps, orders = [], []
    for core in range(NCORES):
        g, r = divmod(core, GSIZE)
        ash_T = adapt[g][r * ASH:(r + 1) * ASH].T            # [D, ASH]
        tabs = np.concatenate([ash_T, constT[g]], axis=1)    # [D, ASH+C]
        tabs_hi, tabs_lo = _bf16_split(tabs)

        e = e_dom[g]
        sel = (e > 0) & (e // ASH == r)
        toks = np.nonzero(sel)[0]
        eloc = e[toks] - r * ASH                             # [0, ASH)
        ntok = ATILES * cap
        evals = np.zeros(ntok, dtype=np.int64)
        order = np.full(ntok, -1, dtype=np.int64)
        for T in range(ATILES):
            tk = toks[(eloc // 128) == T]
            tk = tk[np.argsort(e[tk], kind="stable")]        # HBM row order
            assert tk.size <= cap
            o0 = T * cap
            order[o0:o0 + tk.size] = tk
            evals[o0:o0 + tk.size] = (e[tk] - r * ASH) - T * 128
        ewrap = evals.reshape(ntok // 16, 16).T.astype(np.int16)
        maps.append({
            "tabsH": np.ascontiguousarray(tabs_hi),
            "tabsL": np.ascontiguousarray(tabs_lo),
            "constN": const[g],
            "eidx16": np.ascontiguousarray(np.tile(ewrap, (8, 1))),
        })
        orders.append(order)
    return maps, orders, e_dom, cap


def _run(trace, **inputs):
    maps, orders, e_dom, cap = _in_maps(**inputs)
    nc = _get_nc(cap)
    res = run_bass_kernel_spmd(nc, maps, core_ids=list(range(NCORES)), trace=trace)
    out = np.empty((2, B, S, D), dtype=np.float32)
    for g in range(2):
        rows = np.empty((B * S, D), dtype=np.float32)
        for r in range(GSIZE):
            core = g * GSIZE + r
            # device wrote out[p, j, :] = row of gather position j*128+p
            dev = res.results[core]["out"]                   # [128, ncol, D]
            bypos = dev.transpose(1, 0, 2).reshape(-1, D)    # [ntok, D]
            o = orders[core]
            m = o >= 0
            rows[o[m]] = bypos[m]
        rows[e_dom[g] == 0] = res.results[g * GSIZE]["row0"][0]
        out[g] = rows.reshape(B, S, D)
    return out, res


def kernel(**inputs) -> np.ndarray:
    out, _ = _run(False, **inputs)
    return out


def kernel_traced(**inputs):
    """Returns (out, BassKernelResults-with-exec_time_ns) for test harnesses."""
    return _run(True, **inputs)


# revision 30
# speedup vs baseline: 1.0217x; 1.0217x over previous
"""AdaptiveConstantEmbeddings distributed Bass kernel for one TRN2 chip.

Reference semantics per domain g (two independent domains):
    e        = max(0, idx - C)                       # [B,S] adaptive row ids
    emb      = adapt_table[e]                        # [B,S,D]
    rel      = emb @ const_table.T                   # [B,S,C]
    out[b,s] = const_table rows where rel == rowmax  # top-1 retrieval

Key algebra: rel rows only depend on e, so compute R = adapt @ const.T
once per domain ([A,C]), argmax over C per adaptive row, then
out[b,s] = const_table[best[e[b,s]]] is a pure gather.

v3 changes vs the AllGather-pipelined v2 (230us):
  * ZERO collectives.  v2 AllGather'd the per-core G shards so each core
    could emit its own batches' tokens; the 4 pipelined AGs ran at only
    24-48 GB/s bus and cost ~116us of wall time including a 40us tail.
    v3 instead re-shards the OUTPUT by adaptive-row range: core r of a
    domain group owns rows [r*1024,(r+1)*1024) AND emits exactly the
    domain tokens whose e lands in that range (the host buckets tokens
    per core; e==0 tokens are filled host-side from the exported row0).
    Every gather is then purely local.
  * Matmuls run bf16 3-term hi/lo (R = Ah@Bh + Ah@Bl + Al@Bh; verified 0
    argmax flips) with ONE standalone LDWEIGHTS per 4-8 matmuls
    (weight-block-major), instead of a fused LDWEIGHTS per matmul.
  * Tables stream in over 4 DMA queues (sync/scalar/vector/gpsimd).
  * Per a-tile T: argmax -> indirect-gather const[best] -> g_t[T] in
    DRAM -> token dma_gather for tile T's bucket -> out DMA, all
    pipelined behind the next tiles' matmuls.

Sharding (8 cores, expert-style): cores 0-3 own domain 0, cores 4-7 own
domain 1.  Within a 4-core group the A=4096 adaptive rows split
1024/core for matmul+argmax, and each core outputs the tokens of its
own row range (~2050 of 16384, padded to 8 chunks of CAP).
"""

import numpy as np

from concourse import bacc, bass, mybir, tile
from concourse.bass_utils import run_bass_kernel_spmd

F32 = mybir.dt.float32
BF16 = mybir.dt.bfloat16
I32 = mybir.dt.int32
I16 = mybir.dt.int16
U16 = mybir.dt.uint16

B, S = 16, 1024
C = 4096          # codebook rows per domain
A = 4096          # adaptive rows per domain
D = 256           # embedding dim
NCORES = 8
GSIZE = 4                     # cores per domain group
ASH = A // GSIZE              # 1024 adaptive rows per core
ATILES = ASH // 128           # 8
KCH = D // 128                # 2 contraction chunks
CW = 512                      # psum tile width (one bank per matmul)
CTILES = C // CW              # 8 psum column tiles
CAP0 = 384                    # default tokens per tile-chunk (3*128)

_NC_CACHE = {}


def _build(cap=CAP0, bare=True):
    nc = bacc.Bacc("TRN2", target_bir_lowering=False, debug=False, num_devices=NCORES)

    ncol = ATILES * cap // 128          # out columns (tokens = col*128 + part)

    # hi/lo bf16 split of [adapt_shard.T | const.T]; one DMA per k-chunk
    tabsH = nc.declare_dram_parameter("tabsH", [D, ASH + C], BF16, isOutput=False)
    tabsL = nc.declare_dram_parameter("tabsL", [D, ASH + C], BF16, isOutput=False)
    constN = nc.declare_dram_parameter("constN", [C, D], F32, isOutput=False)
    # wrapped dma_gather indices (tile-local row ids, 0..127):
    # eidx16[q, s] = e''[s*16 + q%16], replicated across the eight
    # 16-partition groups; chunk T uses columns [T*cap/16, (T+1)*cap/16)
    eidx16 = nc.declare_dram_parameter("eidx16", [128, ATILES * cap // 16], I16,
                                       isOutput=False)
    # out[p, j, :] = row of gathered token j*128 + p (host unpermutes)
    out = nc.declare_dram_parameter("out", [128, ncol, D], F32, isOutput=True)
    # G row 0 of this shard (cores 0/4: the row shared by all e==0 tokens)
    row0 = nc.declare_dram_parameter("row0", [1, D], F32, isOutput=True)

    g_t = [nc.dram_tensor(f"g_t{T}", [128, D], F32) for T in range(ATILES)]

    with tile.TileContext(nc) as tc:
        with (
            tc.tile_pool(name="tabs", bufs=1) as tabs_pool,
            tc.tile_pool(name="work", bufs=3) as work,
            tc.tile_pool(name="small", bufs=2) as small,
            tc.tile_pool(name="ps", bufs=8, space="PSUM") as ps,
            tc.tile_pool(name="gather", bufs=2) as gpool,
        ):
            # tabs[hl][k]: [128, ASH+C] bf16
            tabs = [[tabs_pool.tile([128, ASH + C], BF16, name=f"tabs{hl}{k}")
                     for k in range(KCH)] for hl in range(2)]
            srcs = [tabsH, tabsL]
            load_eng = [nc.sync, nc.scalar, nc.gpsimd]
            NQ = len(load_eng)
            e16 = gpool.tile([128, ATILES * cap // 16], I16, name="e16",
                             tag="e16", bufs=1)
            load_insts = [nc.gpsimd.dma_start(e16[:], eidx16[:])]
            li = 1
            # adaptive shard first (lhsT for the first matmuls)
            for hl in range(2):
                for k in range(KCH):
                    load_insts.append(load_eng[li % NQ].dma_start(
                        tabs[hl][k][:, :ASH],
                        srcs[hl][k * 128:(k + 1) * 128, :ASH]))
                    li += 1
            # const bank pairs, both hi+lo per pair before the next pair
            for c in range(CTILES // 2):
                for hl in range(2):
                    for k in range(KCH):
                        load_insts.append(load_eng[li % NQ].dma_start(
                            tabs[hl][k][:, ASH + c * 1024: ASH + (c + 1) * 1024],
                            srcs[hl][k * 128:(k + 1) * 128,
                                     ASH + c * 1024: ASH + (c + 1) * 1024],
                        ))
                        li += 1
            for i in range(NQ, len(load_insts)):
                tile.add_dep_helper(load_insts[i].ins, load_insts[i - NQ].ins,
                                    False, "load order")

            g_insts, o_insts = [], []
            pending_rows = []
            H, L = 0, 1

            for T in range(ATILES):
                psums = [ps.tile([128, CW], F32, name=f"ps{T}_{c}", tag="ps")
                         for c in range(CTILES)]
                # 3-term bf16: Ah@Bh + Ah@Bl + Al@Bh, weight-block-major so
                # consecutive matmuls share lhsT; 4 banks per half so the
                # other half's PSUM copies overlap.
                for half in range(2):
                    cs = range(half * 4, half * 4 + 4)
                    for (hl, k) in ((H, 0), (H, 1), (L, 0), (L, 1)):
                        lhsT = tabs[hl][k][:, T * 128:(T + 1) * 128]
                        rhs_hls = (H, L) if hl == H else (H,)
                        for rhl in rhs_hls:
                            for c in cs:
                                rhs = tabs[rhl][k][:, ASH + c * CW:
                                                   ASH + (c + 1) * CW]
                                st = (hl == H and k == 0 and rhl == H)
                                sp = (hl == L and k == KCH - 1)
                                nc.tensor.matmul(psums[c][:], lhsT=lhsT,
                                                 rhs=rhs, start=st, stop=sp)

                # argmax via independent halves; left wins exact ties, which
                # matches max_index's first-occurrence rule on the full row.
                # The left half's FIND fires as soon as banks 0-3 are copied,
                # overlapping the right half's copies.
                # argmax via independent halves; left wins exact ties, which
                # matches max_index's first-occurrence rule on the full row.
                # The left half's FIND fires as soon as banks 0-3 are copied,
                # overlapping the right half's copies.
                r_sb = work.tile([128, C], F32, name=f"r{T}", tag="r")
                m8all = small.tile([128, 64], F32, name=f"m8a_{T}", tag="m8a")
                m8l = small.tile([128, 8], F32, name=f"m8l_{T}", tag="m8l")
                i8l = small.tile([128, 8], U16, name=f"i8l_{T}", tag="i8l")
                m8r = small.tile([128, 8], F32, name=f"m8r_{T}", tag="m8r")
                i8r = small.tile([128, 8], U16, name=f"i8r_{T}", tag="i8r")
                for c in range(CTILES):
                    nc.scalar.copy(r_sb[:, c * CW:(c + 1) * CW], psums[c][:])
                    nc.vector.max(out=m8all[:, c * 8:(c + 1) * 8],
                                  in_=r_sb[:, c * CW:(c + 1) * CW])
                    if c == 3:
                        nc.vector.max(out=m8l[:], in_=m8all[:, 0:32])
                        nc.vector.max_index(out=i8l[:], in_max=m8l[:],
                                            in_values=r_sb[:, 0:2048])
                nc.vector.max(out=m8r[:], in_=m8all[:, 32:64])
                nc.vector.max_index(out=i8r[:], in_max=m8r[:],
                                    in_values=r_sb[:, 2048:4096])
                best32 = small.tile([128, 1], I32, name=f"b32_{T}", tag="b32")
                nc.vector.tensor_copy(best32[:], i8l[:, :1])
                ir32 = small.tile([128, 1], I32, name=f"ir32_{T}", tag="ir32")
                nc.vector.tensor_copy(ir32[:], i8r[:, :1])
                nc.vector.tensor_scalar(ir32[:], ir32[:], 2048, scalar2=None,
                                        op0=mybir.AluOpType.add)
                rwins = small.tile([128, 1], U16, name=f"rw_{T}", tag="rw")
                nc.vector.tensor_tensor(out=rwins[:], in0=m8l[:, :1],
                                        in1=m8r[:, :1],
                                        op=mybir.AluOpType.is_lt)
                nc.vector.copy_predicated(best32[:], rwins[:], ir32[:])

                # G rows for this tile: const[best[a], :]
                g_tile = small.tile([128, D], F32, name=f"g{T}", tag="g")
                nc.gpsimd.indirect_dma_start(
                    out=g_tile[:],
                    out_offset=None,
                    in_=constN[:, :],
                    in_offset=bass.IndirectOffsetOnAxis(ap=best32[:], axis=0),
                )
                gt_dma = nc.sync.dma_start(g_t[T][:, :], g_tile[:])
                if T == 0:
                    nc.scalar.dma_start(row0[:, :], g_tile[0:1, :])
                # out trigger for the PREVIOUS tile, one tile late so its
                # gather has already landed and the sync queue never blocks
                # (an out trigger emitted with its own tile waits ~5us on the
                # gather and stalls the next g_t write; and the tile
                # scheduler hoists triggers as soon as deps allow, so pin
                # them behind this tile's g_t write instead)
                if pending_rows:
                    Tp, prows = pending_rows.pop(0)
                    oi = nc.sync.dma_start(
                        out[:, Tp * (cap // 128):(Tp + 1) * (cap // 128), :],
                        prows[:])
                    tile.add_dep_helper(oi.ins, gt_dma.ins, False, "o after gt")
                    o_insts.append(oi)

                # token gather for this tile's bucket (pads point at row 0
                # of the tile; host ignores pad positions)
                rows = gpool.tile([128, cap // 128, D], F32,
                                  name=f"rows{T}", tag=f"rows{T}", bufs=1)
                gi = nc.gpsimd.dma_gather(
                    out_ap=rows[:],
                    in_ap=g_t[T][:, :],
                    idxs_ap=e16[:, T * (cap // 16):(T + 1) * (cap // 16)],
                    num_idxs=cap,
                    num_idxs_reg=cap,
                    elem_size=D,
                    single_packet=False,
                )
                if g_insts:
                    tile.add_dep_helper(gi.ins, g_insts[-1].ins, False, "g order")
                g_insts.append(gi)
                pending_rows.append((T, rows))

            # remaining out-DMA triggers (last tile's, on sync)
            for Tp, prows in pending_rows:
                oi = nc.sync.dma_start(
                    out[:, Tp * (cap // 128):(Tp + 1) * (cap // 128), :],
                    prows[:])
                if o_insts:
                    tile.add_dep_helper(oi.ins, o_insts[-1].ins, False, "o order")
                o_insts.append(oi)
    nc.compile()
    return nc


def _get_nc(cap, bare=True):
    key = (cap, bare)
    if key not in _NC_CACHE:
        _NC_CACHE[key] = _build(cap, bare)
    return _NC_CACHE[key]


def _bf16_split(x):
    import ml_dtypes
    hi = x.astype(ml_dtypes.bfloat16)
    lo = (x - hi.astype(np.float32)).astype(ml_dtypes.bfloat16)
    return hi, lo


def _in_maps(idx0, idx1, const_table0, const_table1, adapt_table0, adapt_table1):
    idx = [np.asarray(idx0), np.asarray(idx1)]
    const = [np.ascontiguousarray(np.asarray(const_table0, dtype=np.float32)),
             np.ascontiguousarray(np.asarray(const_table1, dtype=np.float32))]
    adapt = [np.asarray(adapt_table0, dtype=np.float32),
             np.asarray(adapt_table1, dtype=np.float32)]
    constT = [np.ascontiguousarray(c.T) for c in const]
    e_dom = [np.maximum(idx[g].reshape(-1).astype(np.int64) - C, 0)
             for g in range(2)]                       # [B*S] per domain

    # capacity: max tokens in any core's 128-row tile bucket, padded to 128
    cap = CAP0
    for g in range(2):
        nz = e_dom[g][e_dom[g] > 0]
        tc_ = np.bincount(nz // 128, minlength=A // 128)
        need = int(np.ceil(max(tc_.max(), 1) / 128) * 128)
        cap = max(cap, need)

    maps, orders = [], []
    for core in range(NCORES):
        g, r = divmod(core, GSIZE)
        ash_T = adapt[g][r * ASH:(r + 1) * ASH].T            # [D, ASH]
        tabs = np.concatenate([ash_T, constT[g]], axis=1)    # [D, ASH+C]
        tabs_hi, tabs_lo = _bf16_split(tabs)

        e = e_dom[g]
        sel = (e > 0) & (e // ASH == r)
        toks = np.nonzero(sel)[0]
        eloc = e[toks] - r * ASH                             # [0, ASH)
        ntok = ATILES * cap
        evals = np.zeros(ntok, dtype=np.int64)
        order = np.full(ntok, -1, dtype=np.int64)
        for T in range(ATILES):
            tk = toks[(eloc // 128) == T]
            tk = tk[np.argsort(e[tk], kind="stable")]        # HBM row order
            assert tk.size <= cap
            o0 = T * cap
            order[o0:o0 + tk.size] = tk
            evals[o0:o0 + tk.size] = (e[tk] - r * ASH) - T * 128
        ewrap = evals.reshape(ntok // 16, 16).T.astype(np.int16)
        maps.append({
            "tabsH": np.ascontiguousarray(tabs_hi),
            "tabsL": np.ascontiguousarray(tabs_lo),
            "constN": const[g],
            "eidx16": np.ascontiguousarray(np.tile(ewrap, (8, 1))),
        })
        orders.append(order)
    return maps, orders, e_dom, cap


def _run(trace, **inputs):
    maps, orders, e_dom, cap = _in_maps(**inputs)
    nc = _get_nc(cap)
    res = run_bass_kernel_spmd(nc, maps, core_ids=list(range(NCORES)), trace=trace)
    out = np.empty((2, B, S, D), dtype=np.float32)
    for g in range(2):
        rows = np.empty((B * S, D), dtype=np.float32)
        for r in range(GSIZE):
            core = g * GSIZE + r
            # device wrote out[p, j, :] = row of gather position j*128+p
            dev = res.results[core]["out"]                   # [128, ncol, D]
            bypos = dev.transpose(1, 0, 2).reshape(-1, D)    # [ntok, D]
            o = orders[core]
            m = o >= 0
            rows[o[m]] = bypos[m]
        rows[e_dom[g] == 0] = res.results[g * GSIZE]["row0"][0]
        out[g] = rows.reshape(B, S, D)
    return out, res


def kernel(**inputs) -> np.ndarray:
    out, _ = _run(False, **inputs)
    return out


def kernel_traced(**inputs):
    """Returns (out, BassKernelResults-with-exec_time_ns) for test harnesses."""
    return _run(True, **inputs)
